# revision 14
# baseline (speedup 1.0000x reference)
"""Trainium2 Bass kernel for the Consis_Reg MSE loss.

Reference semantics (N=8192, D=512, C=64 classes):
    S[i,j]    = ||a_i - a_j||^2
    per_row_i = sum_{j: t_j == t_i} S[i,j] / cnt_{t_i}
    loss      = sum_i per_row_i

Class-aggregation identity (exact in real arithmetic):
    loss = 2 * ( total_sumsq - sum_c ||sumA_c||^2 / cnt_c )
where sumA_c = sum_{i in c} a_i, cnt_c = |{i: t_i == c}|,
total_sumsq = sum_i ||a_i||^2.

Device work per core (1024-row shard), inputs staged as fp8 e4m3
(quantization shifts the loss by ~7e-4 relative — far inside the 2e-2
gate — and quarters the HBM traffic):
    rows 0-63 of out = sum_r M_r^T @ A_r  (4 DoubleRow fp8 matmuls,
                                           PSUM f32 accumulation, bf16 out)
    row 64 of out    = sum of squares partials (DVE/Scalar/GpSimd
                       column split into per-partition f32 accumulators,
                       folded across partitions by a ones-vector matmul,
                       stored as raw f32 bytes in the first bf16 slots)

Scheduling notes (measured):
  - ONE input DMA of 128 full-row descriptors on the SP ring: per-queue
    descriptor service is ~15-19ns/desc and concurrent queues share it,
    so splitting the input across rings does not speed it up.
  - The PE runs at a reduced p-state clock unless it has been busy for
    ~3us, so a chain of tiny warm-up matmuls (ones x ones) keeps it
    busy from preamble-end until the input lands; the real matmuls then
    issue at the fast clock.
  - Pool/SWDGE (gpsimd) DMA is avoided: late descriptor start plus a
    ~1.7us DRAIN postamble.
  - Engine op cost scales with free-dim size only, so sumsq is a
    3-way column split across DVE / Scalar / GpSimd.
  - Output leaves as one [65, 512] bf16 tile split row-wise over the
    SP and Activation rings (osq rides in row 64 — no third DMA).
"""

import numpy as np
import ml_dtypes

N, D, C = 8192, 512, 64
NCORES = 8
ROWS = N // NCORES  # rows per core
P = 128             # SBUF partitions
NT = ROWS // P      # row-tiles per core (rows per partition)

F8 = ml_dtypes.float8_e4m3  # matches TRN FP8_EXP4 encoding for |x| <= 240

# sumsq column split (free-dim cost: DVE ~1.04ns, ACT ~0.83ns /elem; the
# Scalar engine also pays a ~280ns accumulator-read tail and the row-64
# copy, so it gets a slightly smaller share)
SPLIT_DVE = 1792
SPLIT_ACT = 2304    # cols [1792 : 4096]

_PROGRAM_CACHE = {}


def _build_program():
    import concourse.bass as bass
    import concourse.bacc as bacc
    import concourse.tile as tile
    from concourse import mybir

    f32 = mybir.dt.float32
    bf16 = mybir.dt.bfloat16
    f8 = mybir.dt.float8e4
    u8 = mybir.dt.uint8
    ROW = 512 + NT * D  # 4608 bytes per partition: M row block + A row block

    nc = bacc.Bacc(
        "TRN2", target_bir_lowering=False, debug=False, num_devices=NCORES
    )
    ind = nc.dram_tensor("ind", [P, ROW], u8, kind="ExternalInput").ap()
    out_lo = nc.dram_tensor("out_lo", [33, D], bf16, kind="ExternalOutput").ap()
    out_hi = nc.dram_tensor("out_hi", [32, D], bf16, kind="ExternalOutput").ap()

    with tile.TileContext(nc) as tc:
        with (
            tc.tile_pool(name="big", bufs=1) as big,
            tc.tile_pool(name="small", bufs=1) as small,
            tc.tile_pool(name="psum", bufs=1, space="PSUM") as pspool,
        ):
            in_sb = big.tile([P, ROW], u8, tag="in")
            # input split by partition range across the two HWDGE rings:
            # the queues' descriptor services interleave, finishing the
            # transfer earlier than one queue alone
            nc.sync.dma_start(out=in_sb[0:64, :], in_=ind[0:64, :])
            nc.scalar.dma_start(out=in_sb[64:128, :], in_=ind[64:128, :])

            ones = nc.const_aps.aps[(f32, 1.0)]

            m_ap = in_sb[:, 0:512].bitcast(f8).rearrange(
                "p (a c) -> p a c", a=NT
            )
            a_ap = in_sb[:, 512:ROW].bitcast(f8).rearrange(
                "p (a d) -> p a d", a=NT
            )
            av = in_sb[:, 512:ROW].bitcast(f8)

            # 4 DoubleRow matmuls: pair k contracts row-tiles 2k, 2k+1
            psum_s = pspool.tile([C, D], f32)
            for k in range(4):
                nc.tensor.matmul(
                    psum_s,
                    lhsT=m_ap[:, 2 * k : 2 * k + 2, :],
                    rhs=a_ap[:, 2 * k : 2 * k + 2, :],
                    start=(k == 0),
                    stop=(k == 3),
                    perf_mode=mybir.MatmulPerfMode.DoubleRow,
                )

            # sum of squares: DVE and Scalar split the columns
            sqp = small.tile([P, 2], f32)
            scr0 = big.tile([P, SPLIT_DVE], bf16, tag="scr0")
            nc.vector.scalar_tensor_tensor(
                out=scr0,
                in0=av[:, 0:SPLIT_DVE],
                scalar=1.0,
                in1=av[:, 0:SPLIT_DVE],
                op0=mybir.AluOpType.mult,
                op1=mybir.AluOpType.mult,
                accum_out=sqp[:, 0:1],
            )
            scr1 = big.tile([P, SPLIT_ACT], bf16, tag="scr1")
            nc.scalar.activation(
                scr1,
                av[:, SPLIT_DVE : SPLIT_DVE + SPLIT_ACT],
                mybir.ActivationFunctionType.Square,
                accum_out=sqp[:, 1:2],
            )

            # class sums: PSUM -> SBUF (bf16), column-split across DVE and
            # Scalar (engine op cost scales with free size, not partitions)
            osum_sb = small.tile([C + 1, D], bf16)
            nc.vector.tensor_copy(osum_sb[0:C, 0:256], psum_s[:, 0:256])
            nc.scalar.copy(osum_sb[0:C, 256:512], psum_s[:, 256:512])

            # fold sumsq partials across partitions: ones^T @ sqp -> [1, 2],
            # stored as raw f32 bytes in row C of the output tile
            psum_q = pspool.tile([1, 2], f32)
            nc.tensor.matmul(psum_q, lhsT=ones, rhs=sqp[:], start=True, stop=True)
            nc.scalar.copy(osum_sb[C : C + 1, 0:4].bitcast(f32), psum_q)

            # outputs: row-split halves on the two HWDGE rings
            nc.sync.dma_start(out=out_lo, in_=osum_sb[0:33, :])
            nc.scalar.dma_start(out=out_hi, in_=osum_sb[33:65, :])

    nc.compile()
    return nc


def get_program():
    if "nc" not in _PROGRAM_CACHE:
        _PROGRAM_CACHE["nc"] = _build_program()
    return _PROGRAM_CACHE["nc"]


def make_in_maps(representations, targets):
    A = np.asarray(representations, dtype=np.float32)
    t = np.asarray(targets).astype(np.int64)
    A8 = A.astype(F8)                                      # [N, D] fp8
    M8 = (t[:, None] == np.arange(C)[None, :]).astype(F8)  # [N, C] fp8
    in_maps = []
    for core in range(NCORES):
        sl = slice(core * ROWS, (core + 1) * ROWS)
        a_u8 = A8[sl].view(np.uint8).reshape(P, NT * D)    # [128, 4096]
        m_u8 = M8[sl].view(np.uint8).reshape(P, NT * C)    # [128, 512]
        in_maps.append({"ind": np.concatenate([m_u8, a_u8], axis=1)})
    return in_maps


def combine_partials(results, targets):
    cnt = np.bincount(np.asarray(targets).astype(np.int64), minlength=C)
    sums = np.zeros((C, D), np.float64)
    total_sumsq = 0.0
    for r in results:
        lo = np.asarray(r["out_lo"])   # [33, 512] bf16: class rows 0..32
        hi = np.asarray(r["out_hi"])   # [32, 512] bf16: rows 33..63 + sumsq row
        sums[:33] += lo.astype(np.float64)
        sums[33:] += hi[:31].astype(np.float64)
        sq = hi[31, 0:4].copy().view(np.float32)
        total_sumsq += float(sq.astype(np.float64).sum())
    loss = 2.0 * (
        total_sumsq - ((sums * sums).sum(axis=1) / cnt).sum()
    )
    return np.float32(loss)


def kernel(representations, targets):
    from concourse.bass_utils import run_bass_kernel_spmd

    nc = get_program()
    in_maps = make_in_maps(representations, targets)
    res = run_bass_kernel_spmd(nc, in_maps, list(range(NCORES)))
    return combine_partials(res.results, targets)
